# revision 13
# baseline (speedup 1.0000x reference)
"""Trainium2 Bass kernel for nn_BevEncode (DCNv2-style deformable conv).

Pipeline per NeuronCore (8 cores = 2 batches x 4 group-quarters, 16 groups each):
  P1 conv (PE, fp32r): offset/mask conv3x3 stride2 -> offT2 DRAM, per-g-packed
     channel order so phase 2 can start per 4-group pass.
  P2 select (DVE/ACT): bilinear gather emulated as separable 5x5 hat-weighted
     shifted sums (offsets provably lie in (-2, 2)), mask multiply, grouped
     w_deform contraction via broadcast weights + reduce.

Self-contained: hardcodes shapes for B=2, C=128, H=W=256, G=64, K=9, stride 2.
"""

import sys
import os
import numpy as np

sys.path.insert(0, "/opt/trn_rl_repo")

B, C, H, W = 2, 128, 256, 256
G, KH, KW, KK = 64, 3, 3, 9
HO = WO = 128
GPC = 16            # groups per core
NCORES = 8
NPASS = 4           # conv output passes, 4 groups each
GPP = 4             # groups per pass
COPP = GPP * 27     # 108 conv out-channels per pass (4x18 off then 4x9 mask)
NCHUNK = 16         # conv spatial chunks
CHO = 8             # output rows per chunk
NBANK = CHO * WO // 512
US = [-2, -1, 0, 1, 2]   # y-shift corner candidates (exact; offsets in (-2,2))
VS = [-2, -1, 0, 1, 2]   # x-shift corner candidates
NA = 7              # alpha = ki-1+u in [-3,3]
XRC = 264           # padded col count in XR tiles (4 + 256 + 4)
SLABR, SLABC = 2 * CHO + 1, 258  # conv slab rows/cols (x rows 2ho0-1..+15, cols -1..256)

_PROGRAM_CACHE = {}


def _ceil_div(a, b):
    return (a + b - 1) // b


def build_program(debug_offt=False):
    import concourse.bass as bass
    import concourse.bacc as bacc
    import concourse.tile as tile
    from concourse import mybir

    f32 = mybir.dt.float32
    f32r = mybir.dt.float32r

    nc = bacc.Bacc("TRN2", target_bir_lowering=False, debug=False)

    x_in = nc.dram_tensor("x", [C, H, W], f32, kind="ExternalInput")
    # weights pre-transposed+reordered on host: [ci=128, kk=9, co=432]
    wconv = nc.dram_tensor("wconv", [C, KK, NPASS * COPP], f32, kind="ExternalInput")
    bias_in = nc.dram_tensor("biasv", [NPASS * COPP], f32, kind="ExternalInput")
    wg_in = nc.dram_tensor("wgv", [1, GPC * 36], f32, kind="ExternalInput")
    y_out = nc.dram_tensor("y", [2 * GPC, HO, WO], f32, kind="ExternalOutput")
    offT2 = nc.dram_tensor("offT2", [NPASS * COPP, HO, WO], f32,
                           kind="ExternalOutput" if debug_offt else "Internal")

    def dram_ap(t, off, dims):
        a = t[:]
        return bass.AP(tensor=a.tensor, offset=a.offset + off,
                       ap=[list(d) for d in dims])

    def tile_ap(tt, off, dims):
        a = tt[:]
        return bass.AP(tensor=a.tensor, offset=a.offset + off,
                       ap=[list(a.ap[0])] + [list(d) for d in dims])

    with tile.TileContext(nc) as tc:
        import contextlib
        ctx = contextlib.ExitStack()
        with ctx:
            const_p = ctx.enter_context(tc.tile_pool(name="const", bufs=1))
            slab_p = ctx.enter_context(tc.tile_pool(name="slab", bufs=2))
            convo_p = ctx.enter_context(tc.tile_pool(name="convo", bufs=2))
            psum_p = ctx.enter_context(tc.tile_pool(name="psum", bufs=4, space="PSUM"))
            maps_p = ctx.enter_context(tc.tile_pool(name="maps", bufs=1))
            xr_p = ctx.enter_context(tc.tile_pool(name="xr", bufs=2))
            hat_p = ctx.enter_context(tc.tile_pool(name="hat", bufs=1))
            hy_p = ctx.enter_context(tc.tile_pool(name="hy", bufs=2))
            st1_p = ctx.enter_context(tc.tile_pool(name="st1", bufs=2))
            st2_p = ctx.enter_context(tc.tile_pool(name="st2", bufs=1))
            out_p = ctx.enter_context(tc.tile_pool(name="outb", bufs=2))

            # ---- constants ----
            wsb = const_p.tile([C, KK, NPASS * COPP], f32r)
            nc.gpsimd.dma_start(out=wsb[:], in_=wconv[:])  # cast f32 -> f32r
            bias_sb = const_p.tile([128, NPASS], f32)
            nc.sync.dma_start(
                out=bias_sb[:COPP, :],
                in_=dram_ap(bias_in, 0, [[1, COPP], [COPP, NPASS]]))
            wgt = const_p.tile([128, GPC * 36], f32)
            nc.sync.dma_start(
                out=wgt[:], in_=dram_ap(wg_in, 0, [[0, 128], [1, GPC * 36]]))
            hatc = const_p.tile([128, 6], f32)
            for i, bv in enumerate([2.0, 1.0, 0.0, -1.0, -2.0, 1.0]):
                nc.vector.memset(hatc[:, i:i + 1], bv)

            def conv_pass(p):
                co0 = p * COPP
                for chn in range(NCHUNK):
                    ho0 = chn * CHO
                    slab = slab_p.tile([C, SLABR, SLABC], f32r, tag="slab",
                                       name=f"slab_{p}_{chn}")
                    r0 = 2 * ho0 - 1
                    rlo = max(r0, 0)
                    rn = min(r0 + SLABR, H) - rlo
                    if r0 < 0:
                        nc.vector.memset(slab[:, 0, :].bitcast(f32), 0)
                    nc.vector.memset(slab[:, :, 0].bitcast(f32), 0)
                    nc.vector.memset(slab[:, :, 257].bitcast(f32), 0)
                    nc.gpsimd.dma_start(
                        out=slab[:, rlo - r0:rlo - r0 + rn, 1:257],
                        in_=dram_ap(x_in, rlo * W, [[H * W, C], [W, rn], [1, W]]))
                    convo = convo_p.tile([128, CHO * WO], f32, tag="convo",
                                         name=f"convo_{p}_{chn}")
                    for bank in range(NBANK):
                        ps = psum_p.tile([128, 512], f32, tag="ps",
                                         name=f"ps_{p}_{chn}_{bank}")
                        for kk in range(KK):
                            ki, kj = kk // 3, kk % 3
                            rhs = tile_ap(slab, (8 * bank + ki) * SLABC + kj,
                                          [[2 * SLABC, 4], [2, WO]])
                            nc.tensor.matmul(out=ps[:COPP, :],
                                             lhsT=wsb[:, kk, co0:co0 + COPP],
                                             rhs=rhs,
                                             start=(kk == 0), stop=(kk == KK - 1))
                        nc.vector.tensor_scalar(
                            out=convo[:COPP, bank * 512:(bank + 1) * 512],
                            in0=ps[:COPP, :],
                            scalar1=bias_sb[:COPP, p:p + 1],
                            scalar2=None, op0=mybir.AluOpType.add)
                    nc.sync.dma_start(
                        out=dram_ap(offT2, co0 * HO * WO + ho0 * WO,
                                    [[HO * WO, COPP], [1, CHO * WO]]),
                        in_=convo[:COPP, :])

            def select_group(g, outbuf):
                p = g // GPP
                gl = g % GPP
                co_off = p * COPP + gl * 18
                co_msk = p * COPP + GPP * 18 + gl * 9
                dyt = maps_p.tile([128, KK, WO], f32, tag="dyt", name=f"dyt{g}")
                dxt = maps_p.tile([128, KK, WO], f32, tag="dxt", name=f"dxt{g}")
                mt = maps_p.tile([128, KK, WO], f32, tag="mt", name=f"mt{g}")
                mtr = maps_p.tile([128, KK, WO], f32, tag="mtr", name=f"mtr{g}")
                nc.sync.dma_start(out=dyt[:], in_=dram_ap(
                    offT2, co_off * HO * WO,
                    [[WO, 128], [2 * HO * WO, KK], [1, WO]]))
                nc.sync.dma_start(out=dxt[:], in_=dram_ap(
                    offT2, (co_off + 1) * HO * WO,
                    [[WO, 128], [2 * HO * WO, KK], [1, WO]]))
                nc.sync.dma_start(out=mtr[:], in_=dram_ap(
                    offT2, co_msk * HO * WO,
                    [[WO, 128], [HO * WO, KK], [1, WO]]))
                nc.scalar.activation(out=mt[:], in_=mtr[:],
                                     func=mybir.ActivationFunctionType.Sigmoid,
                                     bias=hatc[:, 2:3], scale=1.0)

                xr = xr_p.tile([128, NA, 2, XRC], f32, tag="xr", name=f"xr{g}")
                nc.vector.memset(tile_ap(xr, 0, [[XRC, NA * 2], [1, 4]]), 0)
                nc.vector.memset(tile_ap(xr, 260, [[XRC, NA * 2], [1, 4]]), 0)
                for a in range(NA):
                    alpha = a - 3
                    plo = max(0, _ceil_div(-alpha, 2))
                    phi = min(127, (255 - alpha) // 2)
                    if plo > 0 or phi < 127:
                        nc.vector.memset(xr[:, a, :, :], 0)
                    for c in range(2):
                        ch = 2 * g + c
                        nc.sync.dma_start(
                            out=xr[plo:phi + 1, a, c, 4:260],
                            in_=dram_ap(x_in, ch * H * W + (2 * plo + alpha) * W,
                                        [[2 * W, phi - plo + 1], [1, W]]))

                hxs = []
                for iv, v in enumerate(VS):
                    t1 = hat_p.tile([128, KK, WO], f32, tag="hs", bufs=2,
                                    name=f"hxs{g}_{v}")
                    hx = hat_p.tile([128, KK, WO], f32, tag=f"hx{v}",
                                    name=f"hx{g}_{v}")
                    nc.scalar.activation(out=t1[:], in_=dxt[:],
                                         func=mybir.ActivationFunctionType.Abs,
                                         bias=hatc[:, iv:iv + 1], scale=1.0)
                    nc.scalar.activation(out=hx[:], in_=t1[:],
                                         func=mybir.ActivationFunctionType.Relu,
                                         bias=hatc[:, 5:6], scale=-1.0)
                    hxs.append(hx)

                tts = [st2_p.tile([128, WO, 18], f32, tag=f"tt{o}",
                                  name=f"tt{o}_{g}") for o in range(2)]
                vals = [st2_p.tile([128, KK, WO], f32, tag=f"val{c}",
                                   name=f"val{g}_{c}") for c in range(2)]
                for iu, u in enumerate(US):
                    hyt = hy_p.tile([128, KK, WO], f32, tag="hyt",
                                    name=f"hyt{g}_{u}")
                    hy = hy_p.tile([128, KK, WO], f32, tag="hy",
                                   name=f"hy{g}_{u}")
                    nc.scalar.activation(out=hyt[:], in_=dyt[:],
                                         func=mybir.ActivationFunctionType.Abs,
                                         bias=hatc[:, iu:iu + 1], scale=1.0)
                    nc.scalar.activation(out=hy[:], in_=hyt[:],
                                         func=mybir.ActivationFunctionType.Relu,
                                         bias=hatc[:, 5:6], scale=-1.0)
                    for c in range(2):
                        val = vals[c]
                        xcu = st1_p.tile([128, KK, WO], f32, tag="xcu",
                                         name=f"xcu{g}_{c}_{u}")
                        for kj in range(3):
                            for iv, v in enumerate(VS):
                                xap = tile_ap(
                                    xr,
                                    (u + 2) * 2 * XRC + c * XRC + (3 + kj + v),
                                    [[2 * XRC, 3], [2, WO]])
                                hxap = tile_ap(hxs[iv], kj * WO,
                                               [[3 * WO, 3], [1, WO]])
                                oap = tile_ap(xcu, kj * WO, [[3 * WO, 3], [1, WO]])
                                if iv == 0:
                                    nc.vector.tensor_tensor(
                                        out=oap, in0=hxap, in1=xap,
                                        op=mybir.AluOpType.mult)
                                else:
                                    tmp = st1_p.tile([128, 3, WO], f32, tag="s1tmp",
                                                     name=f"s1t{g}_{c}_{u}_{kj}_{v}")
                                    nc.vector.tensor_tensor(
                                        out=tmp[:], in0=hxap, in1=xap,
                                        op=mybir.AluOpType.mult)
                                    nc.vector.tensor_tensor(
                                        out=oap, in0=oap, in1=tmp[:],
                                        op=mybir.AluOpType.add)
                        if iu == 0:
                            nc.vector.tensor_tensor(out=val[:], in0=hy[:],
                                                    in1=xcu[:],
                                                    op=mybir.AluOpType.mult)
                        else:
                            tmp2 = st2_p.tile([128, KK, WO], f32, tag="s2tmp",
                                              name=f"s2t{g}_{c}_{u}")
                            nc.vector.tensor_tensor(out=tmp2[:], in0=hy[:],
                                                    in1=xcu[:],
                                                    op=mybir.AluOpType.mult)
                            nc.vector.tensor_tensor(out=val[:], in0=val[:],
                                                    in1=tmp2[:],
                                                    op=mybir.AluOpType.add)
                for c in range(2):
                    val = vals[c]
                    nc.vector.tensor_tensor(out=val[:], in0=val[:], in1=mt[:],
                                            op=mybir.AluOpType.mult)
                    for o in range(2):
                        wgap = tile_ap(wgt, g * 36 + o * 18 + c * 9,
                                       [[1, KK], [0, WO]])
                        oap = tile_ap(tts[o], c * 9, [[1, KK], [18, WO]])
                        nc.vector.tensor_tensor(out=oap, in0=wgap, in1=val[:],
                                                op=mybir.AluOpType.mult)
                for o in range(2):
                    nc.vector.tensor_reduce(
                        out=outbuf[:, 2 * (g % GPP) + o, :],
                        in_=tts[o][:],
                        axis=mybir.AxisListType.X,
                        op=mybir.AluOpType.add)

            for p in range(NPASS):
                conv_pass(p)
                outbuf = out_p.tile([128, 2 * GPP, WO], f32, tag="outbuf",
                                    name=f"outbuf{p}")
                for gl in range(GPP):
                    select_group(p * GPP + gl, outbuf)
                nc.sync.dma_start(
                    out=dram_ap(y_out, p * GPP * 2 * HO * WO,
                                [[WO, 128], [HO * WO, 2 * GPP], [1, WO]]),
                    in_=outbuf[:])

    nc.compile()
    return nc


def _host_prep(inputs):
    x = np.ascontiguousarray(np.asarray(inputs["x"], dtype=np.float32))
    w_offset = np.asarray(inputs["w_offset"], dtype=np.float32)
    b_offset = np.asarray(inputs["b_offset"], dtype=np.float32)
    w_mask = np.asarray(inputs["w_mask"], dtype=np.float32)
    b_mask = np.asarray(inputs["b_mask"], dtype=np.float32)
    w_deform = np.asarray(inputs["w_deform"], dtype=np.float32)

    in_maps = []
    for core in range(NCORES):
        b = core // 4
        q = core % 4
        gs = np.arange(GPC) + q * GPC
        wrows, brows = [], []
        for p in range(NPASS):
            for gl in range(GPP):
                g = gs[p * GPP + gl]
                idx = np.arange(18) + g * KK * 2
                wrows.append(w_offset[idx])
                brows.append(b_offset[idx])
            for gl in range(GPP):
                g = gs[p * GPP + gl]
                idx = np.arange(KK) + g * KK
                wrows.append(w_mask[idx])
                brows.append(b_mask[idx])
        wall = np.concatenate(wrows, axis=0)
        ball = np.ascontiguousarray(np.concatenate(brows, axis=0))
        # rotate input channels so this core's 32 group-channels are first
        perm = np.r_[np.arange(32 * q, C), np.arange(0, 32 * q)]
        wconv = np.ascontiguousarray(
            wall.reshape(432, C, KK)[:, perm, :].transpose(1, 2, 0))
        wg = w_deform.reshape(G, 2, 2, KK)[gs]
        wgv = np.ascontiguousarray(wg.reshape(1, GPC * 36))
        in_maps.append({
            "x": np.ascontiguousarray(x[b][perm]),
            "wconv": wconv,
            "biasv": ball,
            "wgv": wgv,
        })
    return in_maps


def kernel(**inputs):
    from concourse.bass_utils import run_bass_kernel_spmd

    if "prog" not in _PROGRAM_CACHE:
        _PROGRAM_CACHE["prog"] = build_program()
    nc = _PROGRAM_CACHE["prog"]
    in_maps = _host_prep(inputs)
    res = run_bass_kernel_spmd(nc, in_maps, list(range(NCORES)),
                               trace=bool(int(os.environ.get("BEV_TRACE", "0"))))
    _PROGRAM_CACHE["last_result"] = res
    out = np.empty((B, C, HO, WO), dtype=np.float32)
    for core in range(NCORES):
        b = core // 4
        q = core % 4
        out[b, q * 32:(q + 1) * 32] = res.results[core]["y"]
    return out
